# revision 29
# baseline (speedup 1.0000x reference)
"""DeepSet GNN message-passing kernel for 8 TRN2 NeuronCores.

Strategy:
  - segment_ids are sorted, so shard by *segment windows*: 392 windows of 128
    segments, 49 windows per core. Each core handles exactly the neighbor rows
    whose segment falls in its windows -> no cross-core reduction at all.
  - Host folds phi_w2 past the segment sum (segment_sum(h@W2+b2) =
    segment_sum(h)@W2 + counts*b2), transposes neighbors to fp16 [65, N]
    (row 64 = ones for the phi bias) and pads each window's rows to B blocks
    of 128 so all 8 cores run one identical (SPMD) program.
  - Device, per window: ONE big DMA pulls the whole window's X^T columns.
    Per 8-block group, the 8 phi matmuls write disjoint 64-col slices of a
    single PSUM bank, then one batched ACT relu -> fp16 SBUF. Per block, DVE
    builds a one-hot [row, seg] with a single-op is_equal against iota, and
    PE accumulates one_hot.T @ h into a per-window PSUM tile [128 segs, 64].
  - Per window rho tail: PE transpose of pooled S, matmul against
    (phi_w2 @ rho_w1) with a counts row folding the phi_b2 term, ACT relu
    with rho_b1 as per-partition bias, final matmul against rho_w2, DVE
    bias-add of rho_b2 into a resident [2, 6272] output tile; one DMA at end.
"""

import os
import sys

sys.path.insert(0, "/opt/trn_rl_repo")

import numpy as np

LAST_RESULTS = None

N_AGENTS = 50000
N_NEIGH = 1600000
D = 64
N_CORES = 8
SEG_T = 128  # segments per window (= PSUM partition dim)
W_PER_CORE = 49
NW_TOT = N_CORES * W_PER_CORE  # 392 windows of 128 segs >= 50000
SEGS_PER_CORE = W_PER_CORE * SEG_T  # 6272
GRP = 8  # phi blocks per relu batch (1 PSUM bank)


def _build_program(B):
    """Build the SPMD bacc program for B blocks (of 128 rows) per window."""
    from concourse import bacc, mybir
    import concourse.tile as tile

    FP16 = mybir.dt.float16
    F32 = mybir.dt.float32
    Relu = mybir.ActivationFunctionType.Relu
    Copy = mybir.ActivationFunctionType.Copy
    EQ = mybir.AluOpType.is_equal

    NBLK = W_PER_CORE * B
    WCOL = B * 128  # columns per window
    NCOL = NBLK * 128

    nc = bacc.Bacc("TRN2", target_bir_lowering=False, debug=False)
    xta = nc.dram_tensor("xta", [65, NCOL], FP16, kind="ExternalInput").ap()
    qid = nc.dram_tensor("qid", [128, NBLK], FP16, kind="ExternalInput").ap()
    cnt = nc.dram_tensor("cnt", [1, SEGS_PER_CORE], FP16, kind="ExternalInput").ap()
    w1a = nc.dram_tensor("w1a", [65, 64], FP16, kind="ExternalInput").ap()
    waa = nc.dram_tensor("waa", [65, 64], FP16, kind="ExternalInput").ap()
    wba = nc.dram_tensor("wba", [64, 2], FP16, kind="ExternalInput").ap()
    rb1 = nc.dram_tensor("rb1", [64, 1], F32, kind="ExternalInput").ap()
    rb2 = nc.dram_tensor("rb2", [2, 1], F32, kind="ExternalInput").ap()
    iota = nc.dram_tensor("iota", [128, GRP * 128], FP16, kind="ExternalInput").ap()
    iden = nc.dram_tensor("iden", [128, 128], FP16, kind="ExternalInput").ap()
    out = nc.dram_tensor("out", [2, SEGS_PER_CORE], F32, kind="ExternalOutput").ap()

    n_grp = (B + GRP - 1) // GRP
    assert n_grp >= 3, f"rho staging needs >=3 groups per window, got {n_grp}"

    with tile.TileContext(nc) as tc:
        with (
            tc.tile_pool(name="const", bufs=1) as cpool,
            tc.tile_pool(name="x", bufs=3) as xpool,
            tc.tile_pool(name="h", bufs=3) as hspool,
            tc.tile_pool(name="oh", bufs=6) as ohpool,
            tc.tile_pool(name="rho", bufs=2) as rpool,
            tc.tile_pool(name="osb", bufs=1) as opool,
            tc.tile_pool(name="psh", bufs=3, space="PSUM") as psh,
            tc.tile_pool(name="pss", bufs=2, space="PSUM") as pss,
            tc.tile_pool(name="psrho", bufs=2, space="PSUM") as psrho,
        ):
            w1a_t = cpool.tile([65, 64], FP16)
            nc.sync.dma_start(w1a_t[:], w1a[:, :])
            waa_t = cpool.tile([65, 64], FP16)
            nc.sync.dma_start(waa_t[:], waa[:, :])
            wba_t = cpool.tile([64, 2], FP16)
            nc.sync.dma_start(wba_t[:], wba[:, :])
            rb1_t = cpool.tile([64, 1], F32)
            nc.sync.dma_start(rb1_t[:], rb1[:, :])
            rb2_t = cpool.tile([2, 1], F32)
            nc.sync.dma_start(rb2_t[:], rb2[:, :])
            iota_t = cpool.tile([128, GRP * 128], FP16)
            nc.sync.dma_start(iota_t[:], iota[:, :])
            iden_t = cpool.tile([128, 128], FP16)
            nc.sync.dma_start(iden_t[:], iden[:, :])
            cnt_t = cpool.tile([1, SEGS_PER_CORE], FP16)
            nc.sync.dma_start(cnt_t[:], cnt[:, :])
            # all per-block segment ids, loaded once: [128, NBLK] fp16
            qall_t = cpool.tile([128, NBLK], FP16)
            nc.sync.dma_start(qall_t[:], qid[:, :])

            out_sb = opool.tile([2, SEGS_PER_CORE], F32)

            # rho MLP for one window, split into 3 stages so each PE step's
            # cross-engine input has a full block-group of latency to land
            def rho_s1(w, s_ps):
                s_sb = rpool.tile([128, 64], FP16)
                nc.scalar.activation(s_sb[:], s_ps[:], Copy)
                st_ps = psrho.tile([64, 128], FP16, tag="rho")
                nc.tensor.transpose(st_ps[:], s_sb[:], iden_t[:])
                st_sb = rpool.tile([65, 128], FP16)
                nc.scalar.activation(st_sb[0:64, :], st_ps[:], Copy)
                nc.vector.tensor_copy(
                    st_sb[64:65, :], cnt_t[:, SEG_T * w : SEG_T * (w + 1)]
                )
                return st_sb

            def rho_s2(st_sb):
                r_ps = psrho.tile([64, 128], F32, tag="rho")
                nc.tensor.matmul(
                    r_ps[:], lhsT=waa_t[:], rhs=st_sb[:], start=True, stop=True
                )
                r_sb = rpool.tile([64, 128], FP16)
                nc.scalar.activation(r_sb[:], r_ps[:], Relu, bias=rb1_t[:, 0:1])
                return r_sb

            def rho_s3(w, r_sb):
                o_ps = psrho.tile([2, 128], F32, tag="rho")
                nc.tensor.matmul(
                    o_ps[:], lhsT=wba_t[:], rhs=r_sb[:], start=True, stop=True
                )
                nc.vector.tensor_scalar(
                    out=out_sb[:, SEG_T * w : SEG_T * (w + 1)],
                    in0=o_ps[:],
                    scalar1=rb2_t[:, 0:1],
                    scalar2=None,
                    op0=mybir.AluOpType.add,
                )

            s_prev = None
            for w in range(W_PER_CORE):
                xt = xpool.tile([65, WCOL], FP16)
                nc.sync.dma_start(xt[:], xta[:, WCOL * w : WCOL * (w + 1)])
                s_ps = pss.tile([128, 64], F32)
                for g in range(n_grp):
                    nb = min(GRP, B - g * GRP)
                    h_ps = psh.tile([128, 64 * GRP], F32)
                    for k in range(nb):
                        b = g * GRP + k
                        nc.tensor.matmul(
                            h_ps[:, 64 * k : 64 * k + 64],
                            lhsT=xt[:, 128 * b : 128 * b + 128],
                            rhs=w1a_t[:],
                            start=True,
                            stop=True,
                        )
                    hs = hspool.tile([128, 64 * GRP], FP16)
                    nc.scalar.activation(
                        hs[:, : 64 * nb], h_ps[:, : 64 * nb], Relu
                    )
                    # one bulk one-hot build for the whole group of blocks
                    b0 = g * GRP
                    ohb = ohpool.tile([128, GRP * 128], FP16)
                    nc.vector.tensor_tensor(
                        ohb[:, : 128 * nb],
                        iota_t[:, : 128 * nb],
                        qall_t[:, B * w + b0 : B * w + b0 + nb].to_broadcast(
                            [128, nb, 128]
                        ),
                        EQ,
                    )
                    for k in range(nb):
                        b = g * GRP + k
                        nc.tensor.matmul(
                            s_ps[:],
                            lhsT=ohb[:, 128 * k : 128 * k + 128],
                            rhs=hs[:, 64 * k : 64 * k + 64],
                            start=(b == 0),
                            stop=(b == B - 1),
                        )
                    # rho for the previous window, staged across this
                    # window's groups so cross-engine latency hides
                    if s_prev is not None:
                        if g == 0:
                            st_sb_p = rho_s1(w - 1, s_prev)
                        elif g == 1:
                            r_sb_p = rho_s2(st_sb_p)
                        elif g == 2:
                            rho_s3(w - 1, r_sb_p)
                s_prev = s_ps
            st_sb_p = rho_s1(W_PER_CORE - 1, s_prev)
            r_sb_p = rho_s2(st_sb_p)
            rho_s3(W_PER_CORE - 1, r_sb_p)
            nc.sync.dma_start(out[:, :], out_sb[:])
    nc.compile()
    return nc


def _host_prep(neighbors, phi_w1, phi_b1, phi_w2, phi_b2,
               rho_w1, rho_b1, rho_w2, rho_b2, segment_ids):
    ids = np.asarray(segment_ids)
    X = np.asarray(neighbors)

    bounds = np.minimum(np.arange(NW_TOT + 1) * SEG_T, N_AGENTS)
    edges = np.searchsorted(ids, bounds)  # row range per window
    rows_w = np.diff(edges)
    B = int(np.ceil(rows_w.max() / 128))

    NBLK = W_PER_CORE * B
    NCOL = NBLK * 128

    XT = np.ascontiguousarray(X.T).astype(np.float16)  # [64, N]
    counts = np.bincount(ids, minlength=NW_TOT * SEG_T).astype(np.float16)

    in_maps = []
    consts = dict(
        w1a=np.concatenate([phi_w1, phi_b1[None, :]], 0).astype(np.float16),
        waa=np.concatenate(
            [phi_w2 @ rho_w1, (phi_b2 @ rho_w1)[None, :]], 0
        ).astype(np.float16),
        wba=rho_w2.astype(np.float16),
        rb1=rho_b1.reshape(64, 1).astype(np.float32),
        rb2=rho_b2.reshape(2, 1).astype(np.float32),
        iota=np.tile(np.arange(128, dtype=np.float16), (128, GRP)),
        iden=np.eye(128, dtype=np.float16),
    )
    for c in range(N_CORES):
        xta = np.zeros((65, NCOL), np.float16)
        qflat = np.full(NCOL, -1.0, np.float32)
        for wl in range(W_PER_CORE):
            wg = W_PER_CORE * c + wl
            a, e = edges[wg], edges[wg + 1]
            n = e - a
            c0 = wl * B * 128
            xta[0:64, c0 : c0 + n] = XT[:, a:e]
            xta[64, c0 : c0 + n] = 1.0
            qflat[c0 : c0 + n] = (ids[a:e] - SEG_T * wg).astype(np.float32)
        qid = np.ascontiguousarray(qflat.reshape(NBLK, 128).T).astype(
            np.float16
        )  # [128, NBLK]
        cnt = counts[SEGS_PER_CORE * c : SEGS_PER_CORE * (c + 1)].reshape(1, -1)
        in_maps.append(dict(xta=xta, qid=qid, cnt=cnt, **consts))
    return B, in_maps


def kernel(**inputs):
    np_inputs = {k: np.asarray(v) for k, v in inputs.items()}
    B, in_maps = _host_prep(**np_inputs)
    nc = _build_program(B)

    from concourse.bass_utils import run_bass_kernel_spmd

    kw = {}
    if os.environ.get("KERNEL_TRACE"):
        kw = dict(trace=True, tmpdir=os.environ.get("KERNEL_TRACE_DIR") or None)
    res = run_bass_kernel_spmd(nc, in_maps, list(range(N_CORES)), **kw)
    global LAST_RESULTS
    LAST_RESULTS = res
    out_t = np.concatenate(
        [res.results[c]["out"] for c in range(N_CORES)], axis=1
    )  # [2, 50176]
    return np.ascontiguousarray(out_t[:, :N_AGENTS].T).astype(np.float32)


# revision 31
# speedup vs baseline: 1.0666x; 1.0666x over previous
"""DeepSet GNN message-passing kernel for 8 TRN2 NeuronCores.

Strategy:
  - segment_ids are sorted, so shard by *segment windows*: 392 windows of 128
    segments, 49 windows per core. Each core handles exactly the neighbor rows
    whose segment falls in its windows -> no cross-core reduction at all.
  - Host folds phi_w2 past the segment sum (segment_sum(h@W2+b2) =
    segment_sum(h)@W2 + counts*b2), transposes neighbors to fp16 [65, N]
    (row 64 = ones for the phi bias) and pads each window's rows to B blocks
    of 128 so all 8 cores run one identical (SPMD) program.
  - Device, per window: ONE big DMA pulls the whole window's X^T columns.
    Per 8-block group, the 8 phi matmuls write disjoint 64-col slices of a
    single PSUM bank, then one batched ACT relu -> fp16 SBUF. Per block, DVE
    builds a one-hot [row, seg] with a single-op is_equal against iota, and
    PE accumulates one_hot.T @ h into a per-window PSUM tile [128 segs, 64].
  - Per window rho tail: PE transpose of pooled S, matmul against
    (phi_w2 @ rho_w1) with a counts row folding the phi_b2 term, ACT relu
    with rho_b1 as per-partition bias, final matmul against rho_w2, DVE
    bias-add of rho_b2 into a resident [2, 6272] output tile; one DMA at end.
"""

import os
import sys

sys.path.insert(0, "/opt/trn_rl_repo")

import numpy as np

LAST_RESULTS = None

N_AGENTS = 50000
N_NEIGH = 1600000
D = 64
N_CORES = 8
SEG_T = 128  # segments per window (= PSUM partition dim)
W_PER_CORE = 49
NW_TOT = N_CORES * W_PER_CORE  # 392 windows of 128 segs >= 50000
SEGS_PER_CORE = W_PER_CORE * SEG_T  # 6272
GRP = 8  # phi blocks per relu batch (1 PSUM bank)


def _build_program(B):
    """Build the SPMD bacc program for B blocks (of 128 rows) per window."""
    from concourse import bacc, mybir
    import concourse.tile as tile

    FP16 = mybir.dt.float16
    F32 = mybir.dt.float32
    Relu = mybir.ActivationFunctionType.Relu
    Copy = mybir.ActivationFunctionType.Copy
    EQ = mybir.AluOpType.is_equal

    NBLK = W_PER_CORE * B
    WCOL = B * 128  # columns per window
    NCOL = NBLK * 128

    nc = bacc.Bacc("TRN2", target_bir_lowering=False, debug=False)
    xta = nc.dram_tensor("xta", [65, NCOL], FP16, kind="ExternalInput").ap()
    qid = nc.dram_tensor("qid", [128, NBLK], FP16, kind="ExternalInput").ap()
    cnt = nc.dram_tensor("cnt", [1, SEGS_PER_CORE], FP16, kind="ExternalInput").ap()
    w1a = nc.dram_tensor("w1a", [65, 64], FP16, kind="ExternalInput").ap()
    waa = nc.dram_tensor("waa", [65, 64], FP16, kind="ExternalInput").ap()
    wba = nc.dram_tensor("wba", [64, 2], FP16, kind="ExternalInput").ap()
    rb1 = nc.dram_tensor("rb1", [64, 1], F32, kind="ExternalInput").ap()
    rb2 = nc.dram_tensor("rb2", [2, 1], F32, kind="ExternalInput").ap()
    iota = nc.dram_tensor("iota", [128, GRP * 128], FP16, kind="ExternalInput").ap()
    iden = nc.dram_tensor("iden", [128, 128], FP16, kind="ExternalInput").ap()
    out = nc.dram_tensor("out", [2, SEGS_PER_CORE], F32, kind="ExternalOutput").ap()

    n_grp = (B + GRP - 1) // GRP
    assert n_grp >= 3, f"rho staging needs >=3 groups per window, got {n_grp}"

    with tile.TileContext(nc) as tc:
        with (
            tc.tile_pool(name="const", bufs=1) as cpool,
            tc.tile_pool(name="x", bufs=3) as xpool,
            tc.tile_pool(name="h", bufs=7) as hspool,
            tc.tile_pool(name="oh", bufs=7) as ohpool,
            tc.tile_pool(name="rho", bufs=2) as rpool,
            tc.tile_pool(name="osb", bufs=1) as opool,
            tc.tile_pool(name="psh", bufs=3, space="PSUM") as psh,
            tc.tile_pool(name="pss", bufs=2, space="PSUM") as pss,
            tc.tile_pool(name="psrho", bufs=2, space="PSUM") as psrho,
        ):
            w1a_t = cpool.tile([65, 64], FP16)
            nc.sync.dma_start(w1a_t[:], w1a[:, :])
            waa_t = cpool.tile([65, 64], FP16)
            nc.sync.dma_start(waa_t[:], waa[:, :])
            wba_t = cpool.tile([64, 2], FP16)
            nc.sync.dma_start(wba_t[:], wba[:, :])
            rb1_t = cpool.tile([64, 1], F32)
            nc.sync.dma_start(rb1_t[:], rb1[:, :])
            rb2_t = cpool.tile([2, 1], F32)
            nc.sync.dma_start(rb2_t[:], rb2[:, :])
            iota_t = cpool.tile([128, GRP * 128], FP16)
            nc.sync.dma_start(iota_t[:], iota[:, :])
            iden_t = cpool.tile([128, 128], FP16)
            nc.sync.dma_start(iden_t[:], iden[:, :])
            cnt_t = cpool.tile([1, SEGS_PER_CORE], FP16)
            nc.sync.dma_start(cnt_t[:], cnt[:, :])
            # all per-block segment ids, loaded once: [128, NBLK] fp16
            qall_t = cpool.tile([128, NBLK], FP16)
            nc.sync.dma_start(qall_t[:], qid[:, :])

            out_sb = opool.tile([2, SEGS_PER_CORE], F32)

            # rho MLP for one window, split into 3 stages so each PE step's
            # cross-engine input has a full block-group of latency to land
            def rho_s1(w, s_ps):
                s_sb = rpool.tile([128, 64], FP16)
                nc.scalar.activation(s_sb[:], s_ps[:], Copy)
                st_ps = psrho.tile([64, 128], FP16, tag="rho")
                nc.tensor.transpose(st_ps[:], s_sb[:], iden_t[:])
                st_sb = rpool.tile([65, 128], FP16)
                nc.scalar.activation(st_sb[0:64, :], st_ps[:], Copy)
                nc.vector.tensor_copy(
                    st_sb[64:65, :], cnt_t[:, SEG_T * w : SEG_T * (w + 1)]
                )
                return st_sb

            def rho_s2(st_sb):
                r_ps = psrho.tile([64, 128], F32, tag="rho")
                nc.tensor.matmul(
                    r_ps[:], lhsT=waa_t[:], rhs=st_sb[:], start=True, stop=True
                )
                r_sb = rpool.tile([64, 128], FP16)
                nc.scalar.activation(r_sb[:], r_ps[:], Relu, bias=rb1_t[:, 0:1])
                return r_sb

            def rho_s3(w, r_sb):
                o_ps = psrho.tile([2, 128], F32, tag="rho")
                nc.tensor.matmul(
                    o_ps[:], lhsT=wba_t[:], rhs=r_sb[:], start=True, stop=True
                )
                nc.vector.tensor_scalar(
                    out=out_sb[:, SEG_T * w : SEG_T * (w + 1)],
                    in0=o_ps[:],
                    scalar1=rb2_t[:, 0:1],
                    scalar2=None,
                    op0=mybir.AluOpType.add,
                )

            s_prev = None
            for w in range(W_PER_CORE):
                xt = xpool.tile([65, WCOL], FP16)
                nc.sync.dma_start(xt[:], xta[:, WCOL * w : WCOL * (w + 1)])
                s_ps = pss.tile([128, 64], F32)
                # phase 1: all phi matmuls + relus + one-hot builds
                work = []
                for g in range(n_grp):
                    nb = min(GRP, B - g * GRP)
                    h_ps = psh.tile([128, 64 * GRP], F32)
                    for k in range(nb):
                        b = g * GRP + k
                        nc.tensor.matmul(
                            h_ps[:, 64 * k : 64 * k + 64],
                            lhsT=xt[:, 128 * b : 128 * b + 128],
                            rhs=w1a_t[:],
                            start=True,
                            stop=True,
                        )
                    hs = hspool.tile([128, 64 * GRP], FP16)
                    nc.scalar.activation(
                        hs[:, : 64 * nb], h_ps[:, : 64 * nb], Relu
                    )
                    b0 = g * GRP
                    ohb = ohpool.tile([128, GRP * 128], FP16)
                    nc.vector.tensor_tensor(
                        ohb[:, : 128 * nb],
                        iota_t[:, : 128 * nb],
                        qall_t[:, B * w + b0 : B * w + b0 + nb].to_broadcast(
                            [128, nb, 128]
                        ),
                        EQ,
                    )
                    work.append((g, nb, hs, ohb))
                    # rho for the previous window, staged across this
                    # window's groups so cross-engine latency hides
                    if s_prev is not None:
                        if g == 0:
                            st_sb_p = rho_s1(w - 1, s_prev)
                        elif g == 1:
                            r_sb_p = rho_s2(st_sb_p)
                        elif g == 2:
                            rho_s3(w - 1, r_sb_p)
                # phase 2: all segment-sum matmuls in one long PE run
                for g, nb, hs, ohb in work:
                    for k in range(nb):
                        b = g * GRP + k
                        nc.tensor.matmul(
                            s_ps[:],
                            lhsT=ohb[:, 128 * k : 128 * k + 128],
                            rhs=hs[:, 64 * k : 64 * k + 64],
                            start=(b == 0),
                            stop=(b == B - 1),
                        )
                s_prev = s_ps
            st_sb_p = rho_s1(W_PER_CORE - 1, s_prev)
            r_sb_p = rho_s2(st_sb_p)
            rho_s3(W_PER_CORE - 1, r_sb_p)
            nc.sync.dma_start(out[:, :], out_sb[:])
    nc.compile()
    return nc


def _host_prep(neighbors, phi_w1, phi_b1, phi_w2, phi_b2,
               rho_w1, rho_b1, rho_w2, rho_b2, segment_ids):
    ids = np.asarray(segment_ids)
    X = np.asarray(neighbors)

    bounds = np.minimum(np.arange(NW_TOT + 1) * SEG_T, N_AGENTS)
    edges = np.searchsorted(ids, bounds)  # row range per window
    rows_w = np.diff(edges)
    B = int(np.ceil(rows_w.max() / 128))

    NBLK = W_PER_CORE * B
    NCOL = NBLK * 128

    XT = np.ascontiguousarray(X.T).astype(np.float16)  # [64, N]
    counts = np.bincount(ids, minlength=NW_TOT * SEG_T).astype(np.float16)

    in_maps = []
    consts = dict(
        w1a=np.concatenate([phi_w1, phi_b1[None, :]], 0).astype(np.float16),
        waa=np.concatenate(
            [phi_w2 @ rho_w1, (phi_b2 @ rho_w1)[None, :]], 0
        ).astype(np.float16),
        wba=rho_w2.astype(np.float16),
        rb1=rho_b1.reshape(64, 1).astype(np.float32),
        rb2=rho_b2.reshape(2, 1).astype(np.float32),
        iota=np.tile(np.arange(128, dtype=np.float16), (128, GRP)),
        iden=np.eye(128, dtype=np.float16),
    )
    for c in range(N_CORES):
        xta = np.zeros((65, NCOL), np.float16)
        qflat = np.full(NCOL, -1.0, np.float32)
        for wl in range(W_PER_CORE):
            wg = W_PER_CORE * c + wl
            a, e = edges[wg], edges[wg + 1]
            n = e - a
            c0 = wl * B * 128
            xta[0:64, c0 : c0 + n] = XT[:, a:e]
            xta[64, c0 : c0 + n] = 1.0
            qflat[c0 : c0 + n] = (ids[a:e] - SEG_T * wg).astype(np.float32)
        qid = np.ascontiguousarray(qflat.reshape(NBLK, 128).T).astype(
            np.float16
        )  # [128, NBLK]
        cnt = counts[SEGS_PER_CORE * c : SEGS_PER_CORE * (c + 1)].reshape(1, -1)
        in_maps.append(dict(xta=xta, qid=qid, cnt=cnt, **consts))
    return B, in_maps


def kernel(**inputs):
    np_inputs = {k: np.asarray(v) for k, v in inputs.items()}
    B, in_maps = _host_prep(**np_inputs)
    nc = _build_program(B)

    from concourse.bass_utils import run_bass_kernel_spmd

    kw = {}
    if os.environ.get("KERNEL_TRACE"):
        kw = dict(trace=True, tmpdir=os.environ.get("KERNEL_TRACE_DIR") or None)
    res = run_bass_kernel_spmd(nc, in_maps, list(range(N_CORES)), **kw)
    global LAST_RESULTS
    LAST_RESULTS = res
    out_t = np.concatenate(
        [res.results[c]["out"] for c in range(N_CORES)], axis=1
    )  # [2, 50176]
    return np.ascontiguousarray(out_t[:, :N_AGENTS].T).astype(np.float32)
